# revision 77
# baseline (speedup 1.0000x reference)
"""DualPathAttention Trainium2 kernel.

Computes, for each batch row of x [S=512, D=512]:
  global branch: 8-head full self-attention + out-proj
  local branch:  overlapping-window (W=10, stride 5) 4-head attention,
                 scatter-added, + out-proj (folded through the scatter)
  fusion:        relu(concat(global, local) @ fw.T)

Strategy: data-parallel over batch B=32 across 8 NeuronCores (4 batches
per core).  All dense matmuls run in bfloat16 (fp32 PSUM accumulate);
softmax denominators and reciprocals stay fp32 for scale accuracy.

Local attention is decomposed into two block-diagonal phases:
  phase 0 = even windows (starts 0,10,...,510) — aligned 10-token blocks
  phase 1 = odd windows (starts 5,15,...,505) — blocks offset by 5
Each token belongs to exactly one window per phase; the reference's
scatter-add equals (phase0_out + phase1_out), accumulated in PSUM.
Queries are processed in groups of 110 tokens; per-window softmax uses a
block-diagonal mask, exp without max subtraction (scores are ~±1.5), and
denominators via an all-ones stationary matmul (replicated across
partitions) + DVE reciprocal.
"""
import ml_dtypes
import numpy as np

B, S, D = 32, 512, 512
GH, LH = 8, 4
GDH, LDH = D // GH, D // LH          # 64, 128
W, STRIDE = 10, 5
NCORES = 8
BPC = B // NCORES                     # batches per core
GRP = 110                             # local query group size
GROUPS = [(g, min(g + GRP, S)) for g in range(0, S, GRP)]
G_SCALE = 1.0 / np.sqrt(GDH)
L_SCALE = 1.0 / np.sqrt(LDH)

_CACHE = {}
UNSKEW = False


def _win_start(q, phase):
    if phase == 0:
        return 10 * (q // 10)
    if q < 5:
        return None
    return 10 * ((q - 5) // 10) + 5


MASK_M = 512.0   # exact in bf16; exp arg gets -MASK_M*L_SCALE ~ -45 off-block
NWU = 24         # union-band rows (<=22 used)
EPS_BIAS = 230.0  # eps row: exp(-EPS_BIAS*L_SCALE) ~ 1.5e-9 keeps T > 0


def _build_local_consts():
    """Union-band mask factors + per-phase window factors.

    Each query q attends the union of its two windows — a contiguous
    15-key band shared by its 5-query quintet.  exp(scores + mu.T@mv - M)
    realizes the union mask; T = ubT.T @ E gives the per-window softmax
    denominators directly, and C = ub.T (vb / T) merges both phase
    normalizations into a single weighted-AV matmul.  Key row nk is an
    eps seed (included in every window via ub) so T > 0 everywhere."""
    mu = np.zeros((5, NWU, 128), np.float32)
    mv = np.zeros((5, NWU, 4, GRP), np.float32)
    ub = np.zeros((5, 32, 128), np.float32)
    vb = np.zeros((5, 32, 4, GRP), np.float32)
    for g, (q0, q1) in enumerate(GROUPS):
        nq = q1 - q0
        ku0 = max(q0 - 5, 0)
        nk = min(q1 + 5, S) - ku0
        for b in range((nq + 4) // 5):
            qs = q0 + 5 * b
            b0, b1 = max(qs - 5, 0), min(qs + 10, S)
            mu[g, b, b0 - ku0:b1 - ku0] = MASK_M
            mu[g, b, nk] = MASK_M - EPS_BIAS
            for q in range(qs, min(qs + 5, q1)):
                mv[g, b, :, q - q0] = 1.0
        for p in (0, 1):
            starts = sorted({st for q in range(q0, q1)
                             for st in [_win_start(q, p)] if st is not None})
            for wi, st in enumerate(starts):
                ub[g, 16 * p + wi, st - ku0:min(st + W, S) - ku0] = 1.0
            for q in range(q0, q1):
                st = _win_start(q, p)
                if st is None:
                    continue
                vb[g, 16 * p + starts.index(st), :, q - q0] = 1.0
        ub[g, :, nk] = 1.0
    return (mu, mv.reshape(5, NWU, 4 * GRP), ub.transpose(0, 2, 1).copy(),
            ub, vb.reshape(5, 32, 4 * GRP))


def _build_nc(reps=1, debug_taps=False):
    import concourse.bass as bass  # noqa: F401
    import concourse.mybir as mybir
    import concourse.tile as tile
    from concourse import bacc

    F32 = mybir.dt.float32
    F32R = mybir.dt.float32r
    BF16 = mybir.dt.bfloat16
    AF = mybir.ActivationFunctionType

    nc = bacc.Bacc("TRN2", target_bir_lowering=False, debug=False,
                   num_devices=NCORES)

    xT = nc.dram_tensor("xT", [BPC, D, S], BF16, kind="ExternalInput")
    wnames = ["wq_g", "wk_g", "wv_g", "wq_l", "wk_l", "wv_l", "wo_g", "wo_l"]
    wdr = {n: nc.dram_tensor(n, [D, D], BF16, kind="ExternalInput")
           for n in wnames}
    fwT = nc.dram_tensor("fwT", [2 * D, D], BF16, kind="ExternalInput")
    cst = nc.dram_tensor("cst", [128, 128], F32R, kind="ExternalInput")
    cstb = nc.dram_tensor("cstb", [128, 128], BF16, kind="ExternalInput")
    lmask_u = nc.dram_tensor("lmask_u", [5, NWU, 128], BF16,
                             kind="ExternalInput")
    lmask_v = nc.dram_tensor("lmask_v", [5, NWU, 4 * GRP], BF16,
                             kind="ExternalInput")
    lubT = nc.dram_tensor("lubT", [128, 5, 32], BF16, kind="ExternalInput")
    lub = nc.dram_tensor("lub", [32, 5, 128], BF16, kind="ExternalInput")
    lvb = nc.dram_tensor("lvb", [32, 5, 4 * GRP], BF16,
                         kind="ExternalInput")
    out = nc.dram_tensor("out", [BPC, S, D], F32, kind="ExternalOutput")
    dbg = {}
    if debug_taps:
        for n, shp in [("d_lout", [128, 4 * S]), ("d_gout", [128, 4 * S]),
                       ("d_el", [121, 4 * GRP]), ("d_wgt", [120, 4 * GRP]),
                       ("d_vp", [32, 4 * GRP]), ("d_ql", [128, 4 * S]),
                       ("d_vl", [120, S]),
                       ("d_ub", [32, 5 * 128]), ("d_vb", [32, 5 * 4 * GRP]),
                       ("d_ubT", [128, 5 * 32]), ("d_ones", [128, 128]),
                       ("d_mu", [NWU, 5 * 128])]:
            dbg[n] = nc.dram_tensor(n, shp, BF16, kind="ExternalOutput")

    with tile.TileContext(nc) as tc:
        with (
            tc.tile_pool(name="const", bufs=1) as cp,
            tc.tile_pool(name="work", bufs=1) as wp,
            tc.tile_pool(name="pmm", bufs=2, space="PSUM") as pmm,
            tc.tile_pool(name="psc", bufs=2, space="PSUM") as psc,
            tc.tile_pool(name="pav", bufs=2, space="PSUM") as pav,
            tc.tile_pool(name="psmall", bufs=2, space="PSUM") as psmall,
        ):
            # ---------------- constants (first-use DMA order) ----------
            xt0 = wp.tile([128, 4, S], BF16, tag="xt", bufs=2)
            nc.sync.dma_start(
                xt0[:], xT[0].rearrange("(kc p) t -> p kc t", p=128))
            w_sb = {}
            for n in ["wq_g", "wk_g", "wq_l", "wk_l", "wv_g", "wv_l",
                      "wo_g", "wo_l"]:
                t = cp.tile([128, 4, D], BF16, tag=f"w_{n}")
                nc.sync.dma_start(
                    t[:], wdr[n].rearrange("(kc p) n -> p kc n", p=128))
                w_sb[n] = t
            ones_gr = cp.tile([128, 128], F32R, tag="ones_gr")
            nc.sync.dma_start(ones_gr[:], cst[:, :])
            ones_b = cp.tile([128, 128], BF16, tag="ones_b")
            nc.sync.dma_start(ones_b[:], cstb[:, :])
            mu_sb = cp.tile([NWU, 5, 128], BF16, tag="lmask_u")
            nc.sync.dma_start(mu_sb[:], lmask_u.rearrange("g w k -> w g k"))
            mv_sb = cp.tile([NWU, 5, 4 * GRP], BF16, tag="lmask_v")
            nc.sync.dma_start(mv_sb[:], lmask_v.rearrange("g w n -> w g n"))
            ubT_sb = cp.tile([128, 5, 32], BF16, tag="lubT")
            nc.sync.dma_start(ubT_sb[:], lubT[:, :, :])
            ub_sb = cp.tile([32, 5, 128], BF16, tag="lub")
            nc.sync.dma_start(ub_sb[:], lub[:, :, :])
            vb_sb = cp.tile([32, 5, 4 * GRP], BF16, tag="lvb")
            nc.sync.dma_start(vb_sb[:], lvb[:, :, :])
            fw_sb = cp.tile([128, 8, D], BF16, tag="w_fw")
            nc.sync.dma_start(
                fw_sb[:], fwT.rearrange("(kc p) n -> p kc n", p=128))
            zeros20 = cp.tile([128, 20], F32, tag="zeros20")
            nc.vector.memset(zeros20[:], 0.0)
            mbias = cp.tile([128, 1], F32, tag="mbias")
            nc.vector.memset(mbias[:], -MASK_M * L_SCALE)
            # v token-major with per-head ones column: [128, tc, 8, 65];
            # persistent tile — the ones columns are written once
            vg = cp.tile([128, 4, 8, 65], BF16, tag="vg")
            for tcc in range(4):
                nc.vector.tensor_copy(
                    vg[:, tcc, :, 64:65],
                    ones_b[:, 0:8].rearrange("p (h o) -> p h o", h=8))

            def proj_fm(w, xt, tag, on_act=False):
                """Feature-major projection: out[128, 4, S] = w.T-style.
                on_act evacuates via the scalar engine — safe only for the
                early projections, before the exp chain owns that queue."""
                r = wp.tile([128, 4, S], BF16, tag=tag, bufs=2)
                for mc in range(4):
                    ps = pmm.tile([128, S], F32, tag="pmm")
                    for kc in range(4):
                        nc.tensor.matmul(
                            ps[:], w[:, kc, mc * 128:(mc + 1) * 128],
                            xt[:, kc, :], start=(kc == 0), stop=(kc == 3))
                    if on_act:
                        nc.scalar.copy(r[:, mc, :], ps[:])
                    else:
                        nc.vector.tensor_copy(r[:, mc, :], ps[:])
                return r

            def emit_batch(bi, use_xt0=False):
                if use_xt0:
                    xt = xt0
                else:
                    xt = wp.tile([128, 4, S], BF16, tag="xt", bufs=2)
                    nc.sync.dma_start(
                        xt[:], xT[bi].rearrange("(kc p) t -> p kc t", p=128))

                # ---------- global branch ----------
                qg = proj_fm(w_sb["wq_g"], xt, "qfm", on_act=True)
                kg = proj_fm(w_sb["wk_g"], xt, "kfm", on_act=True)
                for tcc in range(4):
                    ps = pmm.tile([128, S], F32, tag="pmm")
                    for kc in range(4):
                        nc.tensor.matmul(
                            ps[:], xt[:, kc, tcc * 128:(tcc + 1) * 128],
                            w_sb["wv_g"][:, kc, :],
                            start=(kc == 0), stop=(kc == 3))
                    nc.scalar.copy(
                        vg[:, tcc, :, 0:64],
                        ps[:].rearrange("p (h e) -> p h e", h=8))

                ql = proj_fm(w_sb["wq_l"], xt, "qfm")
                kl = proj_fm(w_sb["wk_l"], xt, "kfm")

                gout = wp.tile([128, 4, S], BF16, tag="gout")

                def g_norm(prev):
                    # deferred normalization for head h-1: tiny reciprocal
                    # of the PSUM ones-row, fanned out on the idle GpSimd
                    # engine (no PE rep matmul, no [64,512] reciprocal)
                    ps_av, den, th, po = prev
                    denr = wp.tile([1, S], F32R, tag="denr", bufs=2)
                    with nc.allow_low_precision(reason="f32r softmax denom"):
                        nc.vector.reciprocal(denr[0:1, :], ps_av[64:65, :])
                    rg = wp.tile([64, S], F32R, tag="rg", bufs=2)
                    nc.gpsimd.partition_broadcast(rg[:, :], denr[0:1, :],
                                                  channels=64)
                    nc.vector.tensor_mul(
                        gout[po:po + 64, th, :],
                        ps_av[0:64, :], rg[0:64, :])

                gprev = None
                for h in range(GH):
                    th, po = h // 2, 64 * (h % 2)
                    e_tiles = []
                    for kc in range(4):
                        ps_s = psc.tile([128, S], F32, tag="psc")
                        nc.tensor.matmul(
                            ps_s[:],
                            kg[po:po + 64, th, kc * 128:(kc + 1) * 128],
                            qg[po:po + 64, th, :])
                        e = wp.tile([128, S], BF16, tag="gE", bufs=4)
                        nc.scalar.activation(e[:], ps_s[:], AF.Exp,
                                             scale=G_SCALE)
                        e_tiles.append(e)
                    if gprev is not None:
                        g_norm(gprev)
                    ps_av = pav.tile([128, S], F32, tag="pav")
                    for kc in range(4):
                        nc.tensor.matmul(
                            ps_av[0:65, :], vg[:, kc, h, :],
                            e_tiles[kc][:],
                            start=(kc == 0), stop=(kc == 3))
                    gprev = (ps_av, None, th, po)
                g_norm(gprev)

                yg = wp.tile([128, 4, S], BF16, tag="yg")
                for ec in range(4):
                    ps = pmm.tile([128, S], F32, tag="pmm")
                    for kc in range(4):
                        nc.tensor.matmul(
                            ps[:], w_sb["wo_g"][:, kc, ec * 128:(ec + 1) * 128],
                            gout[:, kc, :], start=(kc == 0), stop=(kc == 3))
                    nc.vector.tensor_copy(yg[:, ec, :], ps[:])

                # ---------- local branch ----------
                # one exp over the 15-key union band per query; T = per-
                # window denominators, vp = vb / T in one DVE divide,
                # C = ub.T vp merges both phase normalizations, then a
                # single weighted AV: out = vl.T @ (E * C).  Stages are
                # emitted with a 4-deep group skew so dependent matmuls
                # always have another group's work in front of them.
                lout = wp.tile([128, 4, S], BF16, tag="lout")
                lstate = {}

                def l_geom(g):
                    q0, q1 = GROUPS[g]
                    ku0, ku1 = max(q0 - 5, 0), min(q1 + 5, S)
                    return q0, q1, q1 - q0, ku0, ku1, ku1 - ku0

                def l_s0(g):
                    q0, q1, nq, ku0, ku1, nk = l_geom(g)
                    vl = wp.tile([128, S], BF16, tag="vl", bufs=4)
                    ps_v = pmm.tile([128, S], F32, tag="pmm")
                    for kc in range(4):
                        nc.tensor.matmul(
                            ps_v[0:nk, :], xt[:, kc, ku0:ku1],
                            w_sb["wv_l"][:, kc, :],
                            start=(kc == 0), stop=(kc == 3))
                    nc.vector.tensor_copy(vl[0:nk, :], ps_v[0:nk, :])
                    # scores^T [keys, 4 heads x queries] + union-band mask
                    # (row nk is the eps seed keeping T > 0)
                    ps_ls = psc.tile([128, 4 * GRP], F32, tag="psc")
                    nc.tensor.matmul(
                        ps_ls[0:nk + 1, :], mu_sb[:, g, 0:nk + 1],
                        mv_sb[:, g, :], start=True, stop=False,
                        skip_group_check=True)
                    for h in range(LH):
                        nc.tensor.matmul(
                            ps_ls[0:nk, h * GRP:h * GRP + nq],
                            kl[:, h, ku0:ku1], ql[:, h, q0:q1],
                            start=False, stop=(h == LH - 1),
                            skip_group_check=True)
                    el = wp.tile([128, 4 * GRP], BF16, tag="el", bufs=3)
                    nc.scalar.activation(
                        el[0:nk + 1, :], ps_ls[0:nk + 1, :], AF.Exp,
                        scale=L_SCALE, bias=mbias[0:nk + 1])
                    if debug_taps and bi == 0 and g == 2:
                        nc.sync.dma_start(dbg["d_el"][:, :], el[0:121, :])
                        nc.sync.dma_start(dbg["d_vl"][:, :], vl[0:120, :])
                    lstate[g] = {"vl": vl, "el": el}

                def l_s1(g):
                    _, _, _, _, _, nk = l_geom(g)
                    st = lstate[g]
                    psb = psmall.tile([128, S], F32, tag="psmall")
                    nc.tensor.matmul(psb[0:32, 0:4 * GRP],
                                     ubT_sb[0:nk + 1, g, :],
                                     st["el"][0:nk + 1, :])
                    tr = wp.tile([32, 4 * GRP], F32R, tag="tr", bufs=2)
                    with nc.allow_low_precision(reason="f32r softmax denom"):
                        nc.vector.reciprocal(tr[:, :], psb[0:32, 0:4 * GRP])
                    vp = wp.tile([32, 4 * GRP], BF16, tag="vp", bufs=2)
                    with nc.allow_low_precision(reason="bf16 attn weights"):
                        nc.vector.tensor_mul(vp[:, :], vb_sb[:, g, :],
                                             tr[:, :])
                    if debug_taps and bi == 0 and g == 2:
                        nc.sync.dma_start(dbg["d_vp"][:, :], vp[:, :])
                    st["vp"] = vp

                def l_s2(g):
                    _, _, _, _, _, nk = l_geom(g)
                    st = lstate[g]
                    ps_c = psc.tile([128, 4 * GRP], F32, tag="psc")
                    nc.tensor.matmul(ps_c[0:nk, :], ub_sb[:, g, 0:nk],
                                     st["vp"][:, :])
                    wgt = wp.tile([128, 4 * GRP], BF16, tag="wgt", bufs=2)
                    with nc.allow_low_precision(reason="bf16 attn weights"):
                        nc.vector.tensor_mul(wgt[0:nk, :], ps_c[0:nk, :],
                                             st["el"][0:nk, :])
                    if debug_taps and bi == 0 and g == 2:
                        nc.sync.dma_start(dbg["d_wgt"][:, :], wgt[0:120, :])
                    st["wgt"] = wgt

                def l_s3(g):
                    q0, q1, nq, _, _, nk = l_geom(g)
                    st = lstate.pop(g)
                    ps_lav = pav.tile([128, 4 * GRP], F32, tag="pav")
                    for h in range(LH):
                        nc.tensor.matmul(
                            ps_lav[:, h * GRP:h * GRP + nq],
                            st["vl"][0:nk, h * 128:(h + 1) * 128],
                            st["wgt"][0:nk, h * GRP:h * GRP + nq])
                    for h in range(LH):
                        nc.vector.tensor_copy(lout[:, h, q0:q1],
                                              ps_lav[:, h * GRP:h * GRP + nq])

                NG = len(GROUPS)
                if UNSKEW:
                    for g in range(NG):
                        l_s0(g)
                        l_s1(g)
                        l_s2(g)
                        l_s3(g)
                else:
                    for t in range(NG + 3):
                        if t < NG:
                            l_s0(t)
                        if 0 <= t - 1 < NG:
                            l_s1(t - 1)
                        if 0 <= t - 2 < NG:
                            l_s2(t - 2)
                        if 0 <= t - 3 < NG:
                            l_s3(t - 3)

                if debug_taps and bi == 0:
                    nc.sync.dma_start(
                        dbg["d_ub"][:, :],
                        ub_sb[:].rearrange("p a b -> p (a b)"))
                    nc.sync.dma_start(
                        dbg["d_vb"][:, :],
                        vb_sb[:].rearrange("p a b -> p (a b)"))
                    nc.sync.dma_start(
                        dbg["d_ubT"][:, :],
                        ubT_sb[:].rearrange("p a b -> p (a b)"))
                    nc.sync.dma_start(dbg["d_ones"][:, :], ones_b[:, :])
                    nc.sync.dma_start(
                        dbg["d_mu"][:, :],
                        mu_sb[:].rearrange("p a b -> p (a b)"))
                    nc.sync.dma_start(
                        dbg["d_lout"][:, :],
                        lout[:].rearrange("p c t -> p (c t)"))
                    nc.sync.dma_start(
                        dbg["d_gout"][:, :],
                        gout[:].rearrange("p c t -> p (c t)"))
                    nc.sync.dma_start(
                        dbg["d_ql"][:, :],
                        ql[:].rearrange("p c t -> p (c t)"))

                yl = wp.tile([128, 4, S], BF16, tag="yl")
                for ec in range(4):
                    ps = pmm.tile([128, S], F32, tag="pmm")
                    for kc in range(4):
                        nc.tensor.matmul(
                            ps[:], w_sb["wo_l"][:, kc, ec * 128:(ec + 1) * 128],
                            lout[:, kc, :], start=(kc == 0), stop=(kc == 3))
                    nc.vector.tensor_copy(yl[:, ec, :], ps[:])

                # ---------- fusion ----------
                for tcc in range(4):
                    ps = pmm.tile([128, S], F32, tag="pmm")
                    for fc in range(8):
                        src = yg if fc < 4 else yl
                        nc.tensor.matmul(
                            ps[:], src[:, fc % 4, tcc * 128:(tcc + 1) * 128],
                            fw_sb[:, fc, :], start=(fc == 0), stop=(fc == 7))
                    res = wp.tile([128, S], F32, tag="res")
                    nc.scalar.activation(res[:], ps[:], AF.Relu)
                    nc.sync.dma_start(
                        out[bi, tcc * 128:(tcc + 1) * 128, :], res[:])

            if reps == 1:
                for bi in range(BPC):
                    emit_batch(bi, use_xt0=(bi == 0))
            else:
                # xt0 only carries real data on the first trip; use fresh
                # DMAs inside the loop (timing variant, results unused)
                with tc.For_i(0, reps, 1, hint_engines=(
                        mybir.EngineType.PE, mybir.EngineType.Activation,
                        mybir.EngineType.DVE, mybir.EngineType.SP,
                        mybir.EngineType.Pool)):
                    for bi in range(BPC):
                        emit_batch(bi)

    nc.compile()
    return nc


def host_in_maps(x, gw_in, gw_out, lw_in, lw_out, fw):
    """Per-core input maps: batch-sharded x^T + transposed weights (bf16)."""
    bf = ml_dtypes.bfloat16
    x = np.ascontiguousarray(np.asarray(x, np.float32))
    gw_in = np.asarray(gw_in, np.float32)
    lw_in = np.asarray(lw_in, np.float32)
    consts = {
        "wq_g": np.ascontiguousarray(gw_in[0:D].T).astype(bf),
        "wk_g": np.ascontiguousarray(gw_in[D:2 * D].T).astype(bf),
        "wv_g": np.ascontiguousarray(gw_in[2 * D:3 * D].T).astype(bf),
        "wq_l": np.ascontiguousarray(lw_in[0:D].T).astype(bf),
        "wk_l": np.ascontiguousarray(lw_in[D:2 * D].T).astype(bf),
        "wv_l": np.ascontiguousarray(lw_in[2 * D:3 * D].T).astype(bf),
        "wo_g": np.ascontiguousarray(np.asarray(gw_out, np.float32).T).astype(bf),
        "wo_l": np.ascontiguousarray(np.asarray(lw_out, np.float32).T).astype(bf),
        "fwT": np.ascontiguousarray(np.asarray(fw, np.float32).T).astype(bf),
    }

    mu, mv, ubT, ub, vb = _build_local_consts()
    consts["lmask_u"] = mu.astype(bf)
    consts["lmask_v"] = mv.astype(bf)
    consts["lubT"] = np.ascontiguousarray(ubT.transpose(1, 0, 2)).astype(bf)
    consts["lub"] = np.ascontiguousarray(ub.transpose(1, 0, 2)).astype(bf)
    consts["lvb"] = np.ascontiguousarray(vb.transpose(1, 0, 2)).astype(bf)
    consts["cst"] = np.ones((128, 128), np.float32)
    consts["cstb"] = np.ones((128, 128), bf)

    in_maps = []
    for c in range(NCORES):
        xb = np.ascontiguousarray(
            x[c * BPC:(c + 1) * BPC].transpose(0, 2, 1)).astype(bf)
        in_maps.append({"xT": xb, **consts})
    return in_maps


def kernel(x, gw_in, gb_in, gw_out, gb_out, lw_in, lb_in, lw_out, lb_out,
           fw, fb):
    import sys
    if '/opt/trn_rl_repo' not in sys.path:
        sys.path.insert(0, '/opt/trn_rl_repo')
    from concourse.bass_utils import run_bass_kernel_spmd

    in_maps = host_in_maps(x, gw_in, gw_out, lw_in, lw_out, fw)
    if "nc" not in _CACHE:
        _CACHE["nc"] = _build_nc()
    nc = _CACHE["nc"]
    res = run_bass_kernel_spmd(nc, in_maps, core_ids=list(range(NCORES)))
    return np.concatenate([r["out"] for r in res.results], axis=0)


# revision 86
# speedup vs baseline: 1.1243x; 1.1243x over previous
"""DualPathAttention Trainium2 kernel.

Computes, for each batch row of x [S=512, D=512]:
  global branch: 8-head full self-attention + out-proj
  local branch:  overlapping-window (W=10, stride 5) 4-head attention,
                 scatter-added, + out-proj (folded through the scatter)
  fusion:        relu(concat(global, local) @ fw.T)

Strategy: data-parallel over batch B=32 across 8 NeuronCores (4 batches
per core).  All dense matmuls run in bfloat16 (fp32 PSUM accumulate);
softmax denominators and reciprocals stay fp32 for scale accuracy.

Local attention is decomposed into two block-diagonal phases:
  phase 0 = even windows (starts 0,10,...,510) — aligned 10-token blocks
  phase 1 = odd windows (starts 5,15,...,505) — blocks offset by 5
Each token belongs to exactly one window per phase; the reference's
scatter-add equals (phase0_out + phase1_out), accumulated in PSUM.
Queries are processed in groups of 110 tokens; per-window softmax uses a
block-diagonal mask, exp without max subtraction (scores are ~±1.5), and
denominators via an all-ones stationary matmul (replicated across
partitions) + DVE reciprocal.
"""
import ml_dtypes
import numpy as np

B, S, D = 32, 512, 512
GH, LH = 8, 4
GDH, LDH = D // GH, D // LH          # 64, 128
W, STRIDE = 10, 5
NCORES = 8
BPC = B // NCORES                     # batches per core
GRP = 110                             # local query group size
GROUPS = [(g, min(g + GRP, S)) for g in range(0, S, GRP)]
G_SCALE = 1.0 / np.sqrt(GDH)
L_SCALE = 1.0 / np.sqrt(LDH)

_CACHE = {}
UNSKEW = False


def _win_start(q, phase):
    if phase == 0:
        return 10 * (q // 10)
    if q < 5:
        return None
    return 10 * ((q - 5) // 10) + 5


MASK_M = 512.0   # exact in bf16; exp arg gets -MASK_M*L_SCALE ~ -45 off-block
NWU = 24         # union-band rows (<=22 used)
EPS_BIAS = 230.0  # eps row: exp(-EPS_BIAS*L_SCALE) ~ 1.5e-9 keeps T > 0


def _build_local_consts():
    """Union-band mask factors + per-phase window factors.

    Each query q attends the union of its two windows — a contiguous
    15-key band shared by its 5-query quintet.  exp(scores + mu.T@mv - M)
    realizes the union mask; T = ubT.T @ E gives the per-window softmax
    denominators directly, and C = ub.T (vb / T) merges both phase
    normalizations into a single weighted-AV matmul.  Key row nk is an
    eps seed (included in every window via ub) so T > 0 everywhere."""
    mu = np.zeros((5, NWU, 128), np.float32)
    mv = np.zeros((5, NWU, 4, GRP), np.float32)
    ub = np.zeros((5, 32, 128), np.float32)
    vb = np.zeros((5, 32, 4, GRP), np.float32)
    for g, (q0, q1) in enumerate(GROUPS):
        nq = q1 - q0
        ku0 = max(q0 - 5, 0)
        nk = min(q1 + 5, S) - ku0
        for b in range((nq + 4) // 5):
            qs = q0 + 5 * b
            b0, b1 = max(qs - 5, 0), min(qs + 10, S)
            mu[g, b, b0 - ku0:b1 - ku0] = MASK_M
            mu[g, b, nk] = MASK_M - EPS_BIAS
            for q in range(qs, min(qs + 5, q1)):
                mv[g, b, :, q - q0] = 1.0
        for p in (0, 1):
            starts = sorted({st for q in range(q0, q1)
                             for st in [_win_start(q, p)] if st is not None})
            for wi, st in enumerate(starts):
                ub[g, 16 * p + wi, st - ku0:min(st + W, S) - ku0] = 1.0
            for q in range(q0, q1):
                st = _win_start(q, p)
                if st is None:
                    continue
                vb[g, 16 * p + starts.index(st), :, q - q0] = 1.0
        ub[g, :, nk] = 1.0
    return (mu, mv.reshape(5, NWU, 4 * GRP), ub.transpose(0, 2, 1).copy(),
            ub, vb.reshape(5, 32, 4 * GRP))


def _build_nc(reps=1, debug_taps=False):
    import concourse.bass as bass  # noqa: F401
    import concourse.mybir as mybir
    import concourse.tile as tile
    from concourse import bacc

    F32 = mybir.dt.float32
    F32R = mybir.dt.float32r
    BF16 = mybir.dt.bfloat16
    AF = mybir.ActivationFunctionType

    nc = bacc.Bacc("TRN2", target_bir_lowering=False, debug=False,
                   num_devices=NCORES)

    xT = nc.dram_tensor("xT", [BPC, D, S], BF16, kind="ExternalInput")
    wnames = ["wq_g", "wk_g", "wv_g", "wq_l", "wk_l", "wv_l", "wo_g", "wo_l"]
    wdr = {n: nc.dram_tensor(n, [D, D], BF16, kind="ExternalInput")
           for n in wnames}
    fwT = nc.dram_tensor("fwT", [2 * D, D], BF16, kind="ExternalInput")
    cst = nc.dram_tensor("cst", [128, 128], F32R, kind="ExternalInput")
    cstb = nc.dram_tensor("cstb", [128, 128], BF16, kind="ExternalInput")
    lmask_u = nc.dram_tensor("lmask_u", [5, NWU, 128], BF16,
                             kind="ExternalInput")
    lmask_v = nc.dram_tensor("lmask_v", [5, NWU, 4 * GRP], BF16,
                             kind="ExternalInput")
    lubT = nc.dram_tensor("lubT", [128, 5, 32], BF16, kind="ExternalInput")
    lub = nc.dram_tensor("lub", [32, 5, 128], BF16, kind="ExternalInput")
    lvb = nc.dram_tensor("lvb", [32, 5, 4 * GRP], BF16,
                         kind="ExternalInput")
    out = nc.dram_tensor("out", [BPC, S, D], F32, kind="ExternalOutput")
    dbg = {}
    if debug_taps:
        for n, shp in [("d_lout", [128, 4 * S]), ("d_gout", [128, 4 * S]),
                       ("d_el", [121, 4 * GRP]), ("d_wgt", [120, 4 * GRP]),
                       ("d_vp", [32, 4 * GRP]), ("d_ql", [128, 4 * S]),
                       ("d_vl", [120, S]),
                       ("d_ub", [32, 5 * 128]), ("d_vb", [32, 5 * 4 * GRP]),
                       ("d_ubT", [128, 5 * 32]), ("d_ones", [128, 128]),
                       ("d_mu", [NWU, 5 * 128])]:
            dbg[n] = nc.dram_tensor(n, shp, BF16, kind="ExternalOutput")

    with tile.TileContext(nc) as tc:
        with (
            tc.tile_pool(name="const", bufs=1) as cp,
            tc.tile_pool(name="work", bufs=1) as wp,
            tc.tile_pool(name="pmm", bufs=2, space="PSUM") as pmm,
            tc.tile_pool(name="psc", bufs=2, space="PSUM") as psc,
            tc.tile_pool(name="pav", bufs=2, space="PSUM") as pav,
            tc.tile_pool(name="psmall", bufs=2, space="PSUM") as psmall,
        ):
            # ---------------- constants (first-use DMA order) ----------
            xt0 = wp.tile([128, 4, S], BF16, tag="xt", bufs=2)
            nc.sync.dma_start(
                xt0[:], xT[0].rearrange("(kc p) t -> p kc t", p=128))
            w_sb = {}
            for n in ["wq_g", "wk_g", "wq_l", "wk_l", "wv_g", "wv_l",
                      "wo_g", "wo_l"]:
                t = cp.tile([128, 4, D], BF16, tag=f"w_{n}")
                nc.sync.dma_start(
                    t[:], wdr[n].rearrange("(kc p) n -> p kc n", p=128))
                w_sb[n] = t
            ones_gr = cp.tile([128, 128], F32R, tag="ones_gr")
            nc.sync.dma_start(ones_gr[:], cst[:, :])
            ones_b = cp.tile([128, 128], BF16, tag="ones_b")
            nc.sync.dma_start(ones_b[:], cstb[:, :])
            mu_sb = cp.tile([NWU, 5, 128], BF16, tag="lmask_u")
            nc.sync.dma_start(mu_sb[:], lmask_u.rearrange("g w k -> w g k"))
            mv_sb = cp.tile([NWU, 5, 4 * GRP], BF16, tag="lmask_v")
            nc.sync.dma_start(mv_sb[:], lmask_v.rearrange("g w n -> w g n"))
            ubT_sb = cp.tile([128, 5, 32], BF16, tag="lubT")
            nc.sync.dma_start(ubT_sb[:], lubT[:, :, :])
            ub_sb = cp.tile([32, 5, 128], BF16, tag="lub")
            nc.sync.dma_start(ub_sb[:], lub[:, :, :])
            vb_sb = cp.tile([32, 5, 4 * GRP], BF16, tag="lvb")
            nc.sync.dma_start(vb_sb[:], lvb[:, :, :])
            fw_sb = cp.tile([128, 8, D], BF16, tag="w_fw")
            nc.sync.dma_start(
                fw_sb[:], fwT.rearrange("(kc p) n -> p kc n", p=128))
            zeros20 = cp.tile([128, 20], F32, tag="zeros20")
            nc.vector.memset(zeros20[:], 0.0)
            mbias = cp.tile([128, 1], F32, tag="mbias")
            nc.vector.memset(mbias[:], -MASK_M * L_SCALE)
            # v token-major with per-head ones column: [128, tc, 8, 65];
            # persistent tile — the ones columns are written once
            vg = cp.tile([128, 4, 8, 65], BF16, tag="vg")
            for tcc in range(4):
                nc.vector.tensor_copy(
                    vg[:, tcc, :, 64:65],
                    ones_b[:, 0:8].rearrange("p (h o) -> p h o", h=8))

            def proj_fm(w, xt, tag):
                """Feature-major projection: out[128, 4, S] = w.T-style."""
                r = wp.tile([128, 4, S], BF16, tag=tag, bufs=2)
                for mc in range(4):
                    ps = pmm.tile([128, S], F32, tag="pmm")
                    for kc in range(4):
                        nc.tensor.matmul(
                            ps[:], w[:, kc, mc * 128:(mc + 1) * 128],
                            xt[:, kc, :], start=(kc == 0), stop=(kc == 3))
                    nc.vector.tensor_copy(r[:, mc, :], ps[:])
                return r

            def emit_batch(bi, use_xt0=False):
                if use_xt0:
                    xt = xt0
                else:
                    xt = wp.tile([128, 4, S], BF16, tag="xt", bufs=2)
                    nc.sync.dma_start(
                        xt[:], xT[bi].rearrange("(kc p) t -> p kc t", p=128))

                # ---------- global branch ----------
                qg = proj_fm(w_sb["wq_g"], xt, "qfm")
                kg = proj_fm(w_sb["wk_g"], xt, "kfm")
                for tcc in range(4):
                    ps = pmm.tile([128, S], F32, tag="pmm")
                    for kc in range(4):
                        nc.tensor.matmul(
                            ps[:], xt[:, kc, tcc * 128:(tcc + 1) * 128],
                            w_sb["wv_g"][:, kc, :],
                            start=(kc == 0), stop=(kc == 3))
                    nc.scalar.copy(
                        vg[:, tcc, :, 0:64],
                        ps[:].rearrange("p (h e) -> p h e", h=8))

                ql = proj_fm(w_sb["wq_l"], xt, "qfm")
                kl = proj_fm(w_sb["wk_l"], xt, "kfm")

                gout = wp.tile([128, 4, S], BF16, tag="gout")

                def g_norm(prev):
                    # deferred normalization for head h-1: tiny reciprocal
                    # of the PSUM ones-row, fanned out on the idle GpSimd
                    # engine (no PE rep matmul, no [64,512] reciprocal)
                    ps_av, den, th, po = prev
                    denr = wp.tile([1, S], F32R, tag="denr", bufs=2)
                    with nc.allow_low_precision(reason="f32r softmax denom"):
                        nc.vector.reciprocal(denr[0:1, :], ps_av[64:65, :])
                    rg = wp.tile([64, S], F32R, tag="rg", bufs=2)
                    nc.gpsimd.partition_broadcast(rg[:, :], denr[0:1, :],
                                                  channels=64)
                    nc.vector.tensor_mul(
                        gout[po:po + 64, th, :],
                        ps_av[0:64, :], rg[0:64, :])

                gprev = None
                for h in range(GH):
                    th, po = h // 2, 64 * (h % 2)
                    e_tiles = []
                    for kc in range(4):
                        ps_s = psc.tile([128, S], F32, tag="psc")
                        nc.tensor.matmul(
                            ps_s[:],
                            kg[po:po + 64, th, kc * 128:(kc + 1) * 128],
                            qg[po:po + 64, th, :])
                        e = wp.tile([128, S], BF16, tag="gE", bufs=4)
                        nc.scalar.activation(e[:], ps_s[:], AF.Exp,
                                             scale=G_SCALE)
                        e_tiles.append(e)
                    if gprev is not None:
                        g_norm(gprev)
                    ps_av = pav.tile([128, S], F32, tag="pav")
                    for kc in range(4):
                        nc.tensor.matmul(
                            ps_av[0:65, :], vg[:, kc, h, :],
                            e_tiles[kc][:],
                            start=(kc == 0), stop=(kc == 3))
                    gprev = (ps_av, None, th, po)
                g_norm(gprev)

                yg = wp.tile([128, 4, S], BF16, tag="yg")
                for ec in range(4):
                    ps = pmm.tile([128, S], F32, tag="pmm")
                    for kc in range(4):
                        nc.tensor.matmul(
                            ps[:], w_sb["wo_g"][:, kc, ec * 128:(ec + 1) * 128],
                            gout[:, kc, :], start=(kc == 0), stop=(kc == 3))
                    nc.vector.tensor_copy(yg[:, ec, :], ps[:])

                # ---------- local branch ----------
                # one exp over the 15-key union band per query; T = per-
                # window denominators, vp = vb / T in one DVE divide,
                # C = ub.T vp merges both phase normalizations, then a
                # single weighted AV: out = vl.T @ (E * C).  Stages are
                # emitted with a 4-deep group skew so dependent matmuls
                # always have another group's work in front of them.
                lout = wp.tile([128, 4, S], BF16, tag="lout")
                lstate = {}

                def l_geom(g):
                    q0, q1 = GROUPS[g]
                    ku0, ku1 = max(q0 - 5, 0), min(q1 + 5, S)
                    return q0, q1, q1 - q0, ku0, ku1, ku1 - ku0

                def l_s0(g):
                    q0, q1, nq, ku0, ku1, nk = l_geom(g)
                    vl = wp.tile([128, S], BF16, tag="vl", bufs=4)
                    ps_v = pmm.tile([128, S], F32, tag="pmm")
                    for kc in range(4):
                        nc.tensor.matmul(
                            ps_v[0:nk, :], xt[:, kc, ku0:ku1],
                            w_sb["wv_l"][:, kc, :],
                            start=(kc == 0), stop=(kc == 3))
                    nc.vector.tensor_copy(vl[0:nk, :], ps_v[0:nk, :])
                    # scores^T [keys, 4 heads x queries] + union-band mask
                    # (row nk is the eps seed keeping T > 0)
                    ps_ls = psc.tile([128, 4 * GRP], F32, tag="psc")
                    nc.tensor.matmul(
                        ps_ls[0:nk + 1, :], mu_sb[:, g, 0:nk + 1],
                        mv_sb[:, g, :], start=True, stop=False,
                        skip_group_check=True)
                    for h in range(LH):
                        nc.tensor.matmul(
                            ps_ls[0:nk, h * GRP:h * GRP + nq],
                            kl[:, h, ku0:ku1], ql[:, h, q0:q1],
                            start=False, stop=(h == LH - 1),
                            skip_group_check=True)
                    el = wp.tile([128, 4 * GRP], BF16, tag="el", bufs=3)
                    nc.scalar.activation(
                        el[0:nk + 1, :], ps_ls[0:nk + 1, :], AF.Exp,
                        scale=L_SCALE, bias=mbias[0:nk + 1])
                    if debug_taps and bi == 0 and g == 2:
                        nc.sync.dma_start(dbg["d_el"][:, :], el[0:121, :])
                        nc.sync.dma_start(dbg["d_vl"][:, :], vl[0:120, :])
                    lstate[g] = {"vl": vl, "el": el}

                def l_s1(g):
                    _, _, _, _, _, nk = l_geom(g)
                    st = lstate[g]
                    psb = psmall.tile([128, S], F32, tag="psmall")
                    nc.tensor.matmul(psb[0:32, 0:4 * GRP],
                                     ubT_sb[0:nk + 1, g, :],
                                     st["el"][0:nk + 1, :])
                    tr = wp.tile([32, 4 * GRP], F32R, tag="tr", bufs=2)
                    with nc.allow_low_precision(reason="f32r softmax denom"):
                        nc.vector.reciprocal(tr[:, :], psb[0:32, 0:4 * GRP])
                    vp = wp.tile([32, 4 * GRP], BF16, tag="vp", bufs=2)
                    with nc.allow_low_precision(reason="bf16 attn weights"):
                        nc.vector.tensor_mul(vp[:, :], vb_sb[:, g, :],
                                             tr[:, :])
                    if debug_taps and bi == 0 and g == 2:
                        nc.sync.dma_start(dbg["d_vp"][:, :], vp[:, :])
                    st["vp"] = vp

                def l_s2(g):
                    _, _, _, _, _, nk = l_geom(g)
                    st = lstate[g]
                    ps_c = psc.tile([128, 4 * GRP], F32, tag="psc")
                    nc.tensor.matmul(ps_c[0:nk, :], ub_sb[:, g, 0:nk],
                                     st["vp"][:, :])
                    wgt = wp.tile([128, 4 * GRP], BF16, tag="wgt", bufs=2)
                    with nc.allow_low_precision(reason="bf16 attn weights"):
                        nc.vector.tensor_mul(wgt[0:nk, :], ps_c[0:nk, :],
                                             st["el"][0:nk, :])
                    if debug_taps and bi == 0 and g == 2:
                        nc.sync.dma_start(dbg["d_wgt"][:, :], wgt[0:120, :])
                    st["wgt"] = wgt

                def l_s3(g):
                    q0, q1, nq, _, _, nk = l_geom(g)
                    st = lstate.pop(g)
                    ps_lav = pav.tile([128, 4 * GRP], F32, tag="pav")
                    for h in range(LH):
                        nc.tensor.matmul(
                            ps_lav[:, h * GRP:h * GRP + nq],
                            st["vl"][0:nk, h * 128:(h + 1) * 128],
                            st["wgt"][0:nk, h * GRP:h * GRP + nq])
                    for h in range(LH):
                        nc.vector.tensor_copy(lout[:, h, q0:q1],
                                              ps_lav[:, h * GRP:h * GRP + nq])

                NG = len(GROUPS)
                if UNSKEW:
                    for g in range(NG):
                        l_s0(g)
                        l_s1(g)
                        l_s2(g)
                        l_s3(g)
                else:
                    for t in range(NG + 3):
                        if t < NG:
                            l_s0(t)
                        if 0 <= t - 1 < NG:
                            l_s1(t - 1)
                        if 0 <= t - 2 < NG:
                            l_s2(t - 2)
                        if 0 <= t - 3 < NG:
                            l_s3(t - 3)

                if debug_taps and bi == 0:
                    nc.sync.dma_start(
                        dbg["d_ub"][:, :],
                        ub_sb[:].rearrange("p a b -> p (a b)"))
                    nc.sync.dma_start(
                        dbg["d_vb"][:, :],
                        vb_sb[:].rearrange("p a b -> p (a b)"))
                    nc.sync.dma_start(
                        dbg["d_ubT"][:, :],
                        ubT_sb[:].rearrange("p a b -> p (a b)"))
                    nc.sync.dma_start(dbg["d_ones"][:, :], ones_b[:, :])
                    nc.sync.dma_start(
                        dbg["d_mu"][:, :],
                        mu_sb[:].rearrange("p a b -> p (a b)"))
                    nc.sync.dma_start(
                        dbg["d_lout"][:, :],
                        lout[:].rearrange("p c t -> p (c t)"))
                    nc.sync.dma_start(
                        dbg["d_gout"][:, :],
                        gout[:].rearrange("p c t -> p (c t)"))
                    nc.sync.dma_start(
                        dbg["d_ql"][:, :],
                        ql[:].rearrange("p c t -> p (c t)"))

                yl = wp.tile([128, 4, S], BF16, tag="yl")
                for ec in range(4):
                    ps = pmm.tile([128, S], F32, tag="pmm")
                    for kc in range(4):
                        nc.tensor.matmul(
                            ps[:], w_sb["wo_l"][:, kc, ec * 128:(ec + 1) * 128],
                            lout[:, kc, :], start=(kc == 0), stop=(kc == 3))
                    nc.vector.tensor_copy(yl[:, ec, :], ps[:])

                # ---------- fusion ----------
                for tcc in range(4):
                    ps = pmm.tile([128, S], F32, tag="pmm")
                    for fc in range(8):
                        src = yg if fc < 4 else yl
                        nc.tensor.matmul(
                            ps[:], src[:, fc % 4, tcc * 128:(tcc + 1) * 128],
                            fw_sb[:, fc, :], start=(fc == 0), stop=(fc == 7))
                    res = wp.tile([128, S], F32, tag="res")
                    nc.scalar.activation(res[:], ps[:], AF.Relu)
                    nc.sync.dma_start(
                        out[bi, tcc * 128:(tcc + 1) * 128, :], res[:])

            if reps == 1:
                for bi in range(BPC):
                    emit_batch(bi, use_xt0=(bi == 0))
            else:
                # xt0 only carries real data on the first trip; use fresh
                # DMAs inside the loop (timing variant, results unused)
                with tc.For_i(0, reps, 1, hint_engines=(
                        mybir.EngineType.PE, mybir.EngineType.Activation,
                        mybir.EngineType.DVE, mybir.EngineType.SP,
                        mybir.EngineType.Pool)):
                    for bi in range(BPC):
                        emit_batch(bi)

    nc.compile()
    return nc


def host_in_maps(x, gw_in, gw_out, lw_in, lw_out, fw):
    """Per-core input maps: batch-sharded x^T + transposed weights (bf16)."""
    bf = ml_dtypes.bfloat16
    x = np.ascontiguousarray(np.asarray(x, np.float32))
    gw_in = np.asarray(gw_in, np.float32)
    lw_in = np.asarray(lw_in, np.float32)
    consts = {
        "wq_g": np.ascontiguousarray(gw_in[0:D].T).astype(bf),
        "wk_g": np.ascontiguousarray(gw_in[D:2 * D].T).astype(bf),
        "wv_g": np.ascontiguousarray(gw_in[2 * D:3 * D].T).astype(bf),
        "wq_l": np.ascontiguousarray(lw_in[0:D].T).astype(bf),
        "wk_l": np.ascontiguousarray(lw_in[D:2 * D].T).astype(bf),
        "wv_l": np.ascontiguousarray(lw_in[2 * D:3 * D].T).astype(bf),
        "wo_g": np.ascontiguousarray(np.asarray(gw_out, np.float32).T).astype(bf),
        "wo_l": np.ascontiguousarray(np.asarray(lw_out, np.float32).T).astype(bf),
        "fwT": np.ascontiguousarray(np.asarray(fw, np.float32).T).astype(bf),
    }

    mu, mv, ubT, ub, vb = _build_local_consts()
    consts["lmask_u"] = mu.astype(bf)
    consts["lmask_v"] = mv.astype(bf)
    consts["lubT"] = np.ascontiguousarray(ubT.transpose(1, 0, 2)).astype(bf)
    consts["lub"] = np.ascontiguousarray(ub.transpose(1, 0, 2)).astype(bf)
    consts["lvb"] = np.ascontiguousarray(vb.transpose(1, 0, 2)).astype(bf)
    consts["cst"] = np.ones((128, 128), np.float32)
    consts["cstb"] = np.ones((128, 128), bf)

    in_maps = []
    for c in range(NCORES):
        xb = np.ascontiguousarray(
            x[c * BPC:(c + 1) * BPC].transpose(0, 2, 1)).astype(bf)
        in_maps.append({"xT": xb, **consts})
    return in_maps


def kernel(x, gw_in, gb_in, gw_out, gb_out, lw_in, lb_in, lw_out, lb_out,
           fw, fb):
    import sys
    if '/opt/trn_rl_repo' not in sys.path:
        sys.path.insert(0, '/opt/trn_rl_repo')
    from concourse.bass_utils import run_bass_kernel_spmd

    in_maps = host_in_maps(x, gw_in, gw_out, lw_in, lw_out, fw)
    if "nc" not in _CACHE:
        _CACHE["nc"] = _build_nc()
    nc = _CACHE["nc"]
    res = run_bass_kernel_spmd(nc, in_maps, core_ids=list(range(NCORES)))
    return np.concatenate([r["out"] for r in res.results], axis=0)
